# revision 25
# baseline (speedup 1.0000x reference)
"""Trainium2 Bass kernel: DVS128-gesture spiking CNN inference (batch 8, T=4).

Sharding: data-parallel over batch N=8 -> 1 sample per NeuronCore, weights
replicated; the LIF membrane state lives in SBUF per core so the T-step scan
needs no cross-device traffic.

Per-core network (per timestep): 5x [3x3 SAME conv + BN + LIF + 2x2 maxpool]
then FC(1024->256)+LIF, FC(256->110)+LIF, grouped mean (110->11), accumulated
over T in PSUM.

v2 pipeline (per conv-layer psum tile):
 - DVE STT primes psum with the LIF hard reset of the previous timestep's
   membrane state: pt <- (w < theta_prev) * w  (t=0 uses matmul start=True)
 - PE conv matmuls ACCUMULATE onto the primed psum (start=False), so the
   charge w_t = reset(w_{t-1}) + y_t happens inside the accumulator.
 - ACT drains psum -> w (bf16) with a DE-INTERLEAVED READ (even pixel
   columns first), which is free on the scalar engine and makes every
   downstream DVE op unit-stride.
 - DVE pool stage 1 (max of the two de-interleaved halves), then the spike
   threshold IS_GE on the contiguous q1 (spike-of-max == max-of-spikes),
   then pool stage 2 writes final spikes straight into the next layer's
   zero-padded spad (partition 64 holds the constant-1 bias row).
 - Membrane scaling w_t = 2^t * v_t (weights carry 2^(t-1)) keeps every
   LIF step exact in bf16; thresholds compare against 2^t.
GpSimd does one-time memsets only. No dup1 tile: conv1 runs the standard
9-tap accumulation like layers 2-4.
"""

import numpy as np

C = 64
T = 4
NL = 5
N_CORES = 8
BN_EPS = 1e-5
USE_BF16 = True

RES = [128, 64, 32, 16, 8]  # conv layer input resolution
PAIRED = [True, True, True, False, True]
PF_L = [1024, 1024, 512, 256, 32]  # psum tile free size per layer

_BUILT = {}


def _build_nc(debug=False):
    import concourse.bass as bass  # noqa: F401
    import concourse.mybir as mybir
    import concourse.tile as tile
    from concourse import bacc

    f32 = mybir.dt.float32
    mdt = mybir.dt.bfloat16 if USE_BF16 else f32
    Alu = mybir.AluOpType

    nc = bacc.Bacc(None, target_bir_lowering=False)

    xim_d = nc.dram_tensor("xim", [T, 19, 32 * 4, 130], mdt, kind="ExternalInput")
    w0_d = nc.dram_tensor("conv0T", [19, T * 64], mdt, kind="ExternalInput")
    wl_d = nc.dram_tensor("convsT", [4, 65, T * 576], mdt, kind="ExternalInput")
    c1p_d = nc.dram_tensor("conv1P", [128, T * 192], mdt, kind="ExternalInput")
    c2p_d = nc.dram_tensor("conv2P", [128, T * 192], mdt, kind="ExternalInput")
    f1_d = nc.dram_tensor("fc1k", [128, T * 2048], mdt, kind="ExternalInput")
    f2_d = nc.dram_tensor("fc2k", [128, T * 220], mdt, kind="ExternalInput")
    bb_d = nc.dram_tensor("boostB", [110, 11], f32, kind="ExternalInput")
    out_d = nc.dram_tensor("out", [1, 11], f32, kind="ExternalOutput")

    with tile.TileContext(nc) as tc:
        with (
            tc.tile_pool(name="const", bufs=1) as cpool,
            tc.tile_pool(name="state", bufs=1) as spool,
            tc.tile_pool(name="bands", bufs=8) as bpool,
            tc.tile_pool(name="work", bufs=4) as wpool,
            tc.tile_pool(name="cpsum", bufs=3, space="PSUM") as ppool,
            tc.tile_pool(name="fcpsum", bufs=1, space="PSUM") as pfc,
            tc.tile_pool(name="accpsum", bufs=1, space="PSUM") as pacc,
        ):
            # ---- constants ----
            w0 = cpool.tile([19, T * 64], mdt)
            nc.sync.dma_start(w0[:, :], w0_d[:, :])
            wl = []
            for l in range(1, 5):
                wt = cpool.tile([65, T * 576], mdt, name=f"wl{l}", tag=f"wl{l}")
                nc.sync.dma_start(wt[:, :], wl_d[l - 1])
                wl.append(wt)
            f1 = cpool.tile([128, T * 2048], mdt)
            for k in range(4):
                nc.sync.dma_start(
                    f1[:, k * 2048 : (k + 1) * 2048], f1_d[:, k * 2048 : (k + 1) * 2048]
                )
            f2 = cpool.tile([128, T * 220], mdt)
            nc.sync.dma_start(f2[:, :], f2_d[:, :])
            bb = cpool.tile([110, 11], f32)
            nc.sync.dma_start(bb[:, :], bb_d[:, :])
            c1p = cpool.tile([128, T * 192], mdt)
            nc.sync.dma_start(c1p[:, :], c1p_d[:, :])
            c2p = cpool.tile([128, T * 192], mdt)
            nc.sync.dma_start(c2p[:, :], c2p_d[:, :])

            # ---- state ----
            v = []
            for l in range(5):
                pp = 128 if PAIRED[l] else 64
                g = RES[l] * RES[l] // 2 if PAIRED[l] else RES[l] * RES[l]
                vt = spool.tile([pp, g], mdt, name=f"v{l}", tag=f"v{l}")
                v.append(vt)
            vf1 = spool.tile([128, 2], f32)
            nc.gpsimd.memset(vf1[:, :], 0.0)
            vf2 = spool.tile([110, 1], f32)
            nc.gpsimd.memset(vf2[:, :], 0.0)

            spads = [None]
            for l in range(1, 5):
                hp = RES[l] + 2
                sp = spool.tile([65, hp, hp], mdt, name=f"spad{l}", tag=f"spad{l}")
                nc.gpsimd.memset(sp[0:64, :, :], 0.0)
                nc.gpsimd.memset(sp[64:65, :, :], 1.0)
                spads.append(sp)

            # conv1/2 tap-pair tiles: partitions 0:64 = spad map, 64:128 = the
            # same map shifted one column left (covers dx=0,1 in one K=128
            # matmul); rebuilt per timestep by two SBUF->SBUF DMAs.
            dup1 = spool.tile([128, 66, 66], mdt)
            nc.gpsimd.memset(dup1[:, :, :], 0.0)
            dup2 = spool.tile([128, 34, 34], mdt)
            nc.gpsimd.memset(dup2[:, :, :], 0.0)
            dups = [None, dup1, dup2]
            cps = [None, c1p, c2p]

            s4p = spool.tile([128, 8], mdt)
            s1 = spool.tile([128, 2], mdt)
            s2 = spool.tile([110, 1], f32)

            acc_ps = pacc.tile([1, 11], f32)

            bands = {}

            def emit_dup(t, l):
                dp = dups[l]
                hp = RES[l] + 2
                nc.sync.dma_start(dp[0:64, :, :], spads[l][0:64, :, :])
                nc.sync.dma_start(dp[64:128, :, 0 : hp - 1], spads[l][0:64, :, 1:hp])

            def emit_bands(t):
                bs = []
                for bi in range(4):
                    bt = bpool.tile([19, 32, 130], mdt, name=f"band{bi}", tag="band")
                    nc.sync.dma_start(bt[0:10, :, :], xim_d[t, 0:10, bi * 32 : (bi + 1) * 32, :])
                    nc.sync.dma_start(bt[10:19, :, :], xim_d[t, 10:19, bi * 32 : (bi + 1) * 32, :])
                    bs.append(bt)
                bands[t] = bs

            def emit_layer(t, l):
                W = RES[l]
                paired = PAIRED[l]
                PF = PF_L[l]
                halfH = W // 2 if paired else W
                G = W * W // 2 if paired else W * W
                pp = 128 if paired else 64
                ntiles = G // PF
                rows_tile = PF // W
                banks = (PF + 511) // 512
                W2 = W // 2
                R2 = rows_tile // 2
                H2 = W // 2
                theta = float(2 ** t)
                theta_p = float(2 ** (t - 1))

                for ti in range(ntiles):
                    pt = ppool.tile([pp, PF], f32, name="cps", tag="cps")
                    vv = v[l][:, ti * PF : (ti + 1) * PF]
                    # view of vv as (two, x) and of pt in matching iteration
                    # order (psum natural pixel order; vv stores even pixels
                    # then odd pixels of each tile)
                    vv3 = vv.rearrange("p (two x) -> p two x", two=2)
                    pt3 = pt.rearrange("p (x two) -> p two x", two=2)
                    if t > 0:
                        # LIF hard reset of w_{t-1} primed into psum
                        nc.vector.scalar_tensor_tensor(
                            pt3[0:pp], vv3[0:pp], theta_p, vv3[0:pp], Alu.is_lt, Alu.mult
                        )
                    for b in range(banks):
                        cw = min(512, PF - b * 512)
                        rows_cw = cw // W
                        for half in range(2 if paired else 1):
                            r0 = (half * halfH if paired else 0) + ti * rows_tile + b * (512 // W)
                            oap = pt[64 * half : 64 * half + 64, b * 512 : b * 512 + cw]
                            if l == 0:
                                bi, r_loc = divmod(r0, 32)
                                nc.tensor.matmul(
                                    oap,
                                    w0[:, t * 64 : (t + 1) * 64],
                                    bands[t][bi][0:19, r_loc : r_loc + rows_cw, 0:128],
                                    start=(t == 0),
                                    stop=True,
                                    skip_group_check=True,
                                )
                            elif l <= 2:
                                for pi in range(6):
                                    if pi < 3:  # tap pair (dy,0)+(dy,1) via dup tile
                                        dy = pi
                                        nc.tensor.matmul(
                                            oap,
                                            cps[l][0:128, t * 192 + pi * 64 : t * 192 + pi * 64 + 64],
                                            dups[l][0:128, r0 + dy : r0 + dy + rows_cw, 0:W],
                                            start=(t == 0 and pi == 0),
                                            stop=False,
                                            skip_group_check=True,
                                        )
                                    else:  # singles (dy,2); last carries bias row
                                        dy = pi - 3
                                        p = dy * 3 + 2
                                        kp = 65 if p == 8 else 64
                                        nc.tensor.matmul(
                                            oap,
                                            wl[l - 1][0:kp, t * 576 + p * 64 : t * 576 + p * 64 + 64],
                                            spads[l][0:kp, r0 + dy : r0 + dy + rows_cw, 2 : 2 + W],
                                            start=False,
                                            stop=(pi == 5),
                                            skip_group_check=True,
                                        )
                            else:
                                for p in range(9):
                                    dy, dx = divmod(p, 3)
                                    kp = 65 if p == 8 else 64
                                    nc.tensor.matmul(
                                        oap,
                                        wl[l - 1][0:kp, t * 576 + p * 64 : t * 576 + p * 64 + 64],
                                        spads[l][0:kp, r0 + dy : r0 + dy + rows_cw, dx : dx + W],
                                        start=(t == 0 and p == 0),
                                        stop=(p == 8),
                                        skip_group_check=True,
                                    )
                    # ACT drain: w_t <- psum (de-interleaved read, unit write)
                    nc.scalar.copy(vv3[0:pp], pt3[0:pp])
                    # pool stage 1: max over pixel pairs == max of halves
                    q1 = wpool.tile([pp, PF // 2], mdt, name="q1", tag="q1")
                    nc.vector.tensor_tensor(
                        q1[:, :], vv[:, 0 : PF // 2], vv[:, PF // 2 : PF], Alu.max
                    )
                    # spike threshold on the contiguous q1 (in place)
                    nc.vector.tensor_scalar(q1[:, :], q1[:, :], theta, None, Alu.is_ge)
                    # pool stage 2: max over row pairs -> spikes into spad
                    q1r = q1.rearrange("p (r two w) -> p r two w", two=2, w=W2)
                    pr0 = ti * R2
                    if l < 4:
                        spn = spads[l + 1]
                        nc.vector.tensor_tensor(
                            spn[0:64, 1 + pr0 : 1 + pr0 + R2, 1 : 1 + W2],
                            q1r[0:64, :, 0, :], q1r[0:64, :, 1, :], Alu.max,
                        )
                        if paired:
                            hb = H2 // 2
                            nc.vector.tensor_tensor(
                                spn[0:64, 1 + hb + pr0 : 1 + hb + pr0 + R2, 1 : 1 + W2],
                                q1r[64:128, :, 0, :], q1r[64:128, :, 1, :], Alu.max,
                            )
                    else:
                        s4r = s4p.rearrange("p (r w) -> p r w", w=4)
                        nc.vector.tensor_tensor(
                            s4r[0:64, :, :], q1r[0:64, :, 0, :], q1r[0:64, :, 1, :], Alu.max
                        )
                        nc.vector.tensor_tensor(
                            s4r[64:128, :, :], q1r[64:128, :, 0, :], q1r[64:128, :, 1, :], Alu.max
                        )

            def emit_fc(t):
                theta = float(2 ** t)
                ps1 = pfc.tile([128, 2], f32, name="fcps", tag="fcps")
                for h in range(2):
                    for f in range(8):
                        nc.tensor.matmul(
                            ps1[0:128, h : h + 1],
                            f1[:, t * 2048 + (f * 2 + h) * 128 : t * 2048 + (f * 2 + h + 1) * 128],
                            s4p[:, f : f + 1],
                            start=(f == 0),
                            stop=(f == 7),
                        )
                nc.vector.scalar_tensor_tensor(
                    vf1[:, :], vf1[:, :], 1.0, ps1[0:128, 0:2], Alu.mult, Alu.add
                )
                nc.vector.tensor_scalar(s1[:, :], vf1[:, :], theta, None, Alu.is_ge)
                nc.vector.scalar_tensor_tensor(
                    vf1[:, :], vf1[:, :], theta, vf1[:, :], Alu.is_lt, Alu.mult
                )

                ps2 = pfc.tile([110, 1], f32, name="fcps2", tag="fcps")
                for h in range(2):
                    nc.tensor.matmul(
                        ps2[0:110, 0:1],
                        f2[:, t * 220 + h * 110 : t * 220 + (h + 1) * 110],
                        s1[:, h : h + 1],
                        start=(h == 0),
                        stop=(h == 1),
                    )
                nc.vector.scalar_tensor_tensor(
                    vf2[:, :], vf2[:, :], 1.0, ps2[0:110, 0:1], Alu.mult, Alu.add
                )
                nc.vector.tensor_scalar(s2[:, :], vf2[:, :], theta, None, Alu.is_ge)
                nc.vector.scalar_tensor_tensor(
                    vf2[:, :], vf2[:, :], theta, vf2[:, :], Alu.is_lt, Alu.mult
                )
                nc.tensor.matmul(
                    acc_ps[0:1, 0:11],
                    s2[0:110, 0:1],
                    bb[0:110, 0:11],
                    start=(t == 0),
                    stop=(t == T - 1),
                )

            emit_bands(0)
            emit_layer(0, 0)
            emit_dup(0, 1)
            emit_layer(0, 1)
            emit_dup(0, 2)
            for t in range(T):
                if t + 1 < T:
                    emit_bands(t + 1)
                    emit_layer(t + 1, 0)
                    emit_dup(t + 1, 1)
                for l in range(2, 5):
                    emit_layer(t, l)
                emit_fc(t)
                if t + 1 < T:
                    emit_layer(t + 1, 1)
                    emit_dup(t + 1, 2)

            out_sb = spool.tile([1, 11], f32)
            nc.scalar.copy(out_sb[:, :], acc_ps[0:1, 0:11])
            nc.sync.dma_start(out_d[0:1, 0:11], out_sb[:, :])

            if debug:
                dv0 = nc.dram_tensor("dbg_v0", [128, 8192], mdt, kind="ExternalOutput")
                nc.sync.dma_start(dv0[:, :], v[0][:, :])
                dv1 = nc.dram_tensor("dbg_v1", [128, 2048], mdt, kind="ExternalOutput")
                nc.sync.dma_start(dv1[:, :], v[1][:, :])
                dsp = nc.dram_tensor("dbg_spad1", [65, 66, 66], mdt, kind="ExternalOutput")
                nc.sync.dma_start(dsp[:, :, :], spads[1][:, :, :])

    nc.compile()
    return nc


def _prep_host(x, conv0_w, convs_w, bn_gamma, bn_beta, bn_mean, bn_var, fc1_w, fc2_w):
    f32 = np.float32
    x = np.asarray(x, f32)
    conv0_w = np.asarray(conv0_w, f32)
    convs_w = np.asarray(convs_w, f32)
    g = np.asarray(bn_gamma, f32) / np.sqrt(np.asarray(bn_var, f32) + BN_EPS)
    bconst = np.asarray(bn_beta, f32) - np.asarray(bn_mean, f32) * g
    fc1_w = np.asarray(fc1_w, f32)
    fc2_w = np.asarray(fc2_w, f32)

    n = x.shape[0]
    ts_scale = np.array([2.0 ** (t - 1) for t in range(T)], f32)

    conv0T = np.zeros((19, T * 64), f32)
    convsT = np.zeros((4, 65, T * 576), f32)
    conv1P = np.zeros((128, T * 192), f32)
    conv2P = np.zeros((128, T * 192), f32)
    for t in range(T):
        sc = ts_scale[t]
        c0 = slice(t * 64, (t + 1) * 64)
        for p in range(9):
            dy, dx = divmod(p, 3)
            for ci in range(2):
                conv0T[2 * p + ci, c0] = sc * g[0] * conv0_w[:, ci, dy, dx]
        conv0T[18, c0] = sc * bconst[0]
        for l in range(1, 5):
            for p in range(9):
                dy, dx = divmod(p, 3)
                convsT[l - 1, 0:64, t * 576 + p * 64 : t * 576 + (p + 1) * 64] = (
                    sc * g[l][None, :] * convs_w[l - 1][:, :, dy, dx].T
                )
            convsT[l - 1, 64, t * 576 + 8 * 64 : t * 576 + 9 * 64] = sc * bconst[l]
        for dy in range(3):
            for li, cpx in ((1, conv1P), (2, conv2P)):
                cpx[0:64, t * 192 + dy * 64 : t * 192 + (dy + 1) * 64] = (
                    sc * g[li][None, :] * convs_w[li - 1][:, :, dy, 0].T
                )
                cpx[64:128, t * 192 + dy * 64 : t * 192 + (dy + 1) * 64] = (
                    sc * g[li][None, :] * convs_w[li - 1][:, :, dy, 1].T
                )

    xpad = np.zeros((n, T, 2, 130, 130), f32)
    xpad[:, :, :, 1:129, 1:129] = x
    xim = np.zeros((n, T, 19, 130, 130), f32)
    for p in range(9):
        dy, dx = divmod(p, 3)
        for ci in range(2):
            xim[:, :, 2 * p + ci, 0:128, 0:128] = xpad[:, :, ci, dy : dy + 128, dx : dx + 128]
    xim[:, :, 18] = 1.0
    xim = np.ascontiguousarray(xim[:, :, :, 0:128, :])

    p_idx = np.arange(128)
    fc1k = np.zeros((128, T * 2048), f32)
    fc2k = np.zeros((128, T * 220), f32)
    for t in range(T):
        sc = ts_scale[t]
        for f in range(8):
            kcol = (p_idx % 64) * 16 + (p_idx // 64) * 8 + f
            for h in range(2):
                fc1k[:, t * 2048 + (f * 2 + h) * 128 : t * 2048 + (f * 2 + h + 1) * 128] = (
                    sc * fc1_w[h * 128 : (h + 1) * 128, kcol].T
                )
        for h in range(2):
            fc2k[:, t * 220 + h * 110 : t * 220 + (h + 1) * 110] = (
                sc * fc2_w[:, h * 128 : (h + 1) * 128].T
            )

    boostB = np.zeros((110, 11), f32)
    for k in range(110):
        boostB[k, k // 10] = 0.1

    if USE_BF16:
        import ml_dtypes

        bf16 = ml_dtypes.bfloat16
        xim, conv0T, convsT, conv1P, conv2P, fc1k, fc2k = (
            a.astype(bf16) for a in (xim, conv0T, convsT, conv1P, conv2P, fc1k, fc2k)
        )
    return xim, conv0T, convsT, conv1P, conv2P, fc1k, fc2k, boostB


def kernel(**inputs):
    import os

    from concourse.bass_utils import run_bass_kernel_spmd

    debug = bool(int(os.environ.get("KERNEL_DEBUG", "0")))

    x = np.asarray(inputs["x"], np.float32)
    assert x.shape == (8, 4, 2, 128, 128), x.shape
    xim, conv0T, convsT, conv1P, conv2P, fc1k, fc2k, boostB = _prep_host(
        x,
        inputs["conv0_w"],
        inputs["convs_w"],
        inputs["bn_gamma"],
        inputs["bn_beta"],
        inputs["bn_mean"],
        inputs["bn_var"],
        inputs["fc1_w"],
        inputs["fc2_w"],
    )

    if debug not in _BUILT:
        _BUILT[debug] = _build_nc(debug)
    nc = _BUILT[debug]

    shared = dict(conv0T=conv0T, convsT=convsT, conv1P=conv1P, conv2P=conv2P, fc1k=fc1k, fc2k=fc2k, boostB=boostB)
    in_maps = [dict(xim=np.ascontiguousarray(xim[n]), **shared) for n in range(N_CORES)]
    res = run_bass_kernel_spmd(nc, in_maps, core_ids=list(range(N_CORES)))
    global LAST_RESULT
    LAST_RESULT = res
    return np.stack([res.results[n]["out"][0] for n in range(N_CORES)], axis=0)


# revision 28
# speedup vs baseline: 1.0121x; 1.0121x over previous
"""Trainium2 Bass kernel: DVS128-gesture spiking CNN inference (batch 8, T=4).

Sharding: data-parallel over batch N=8 -> 1 sample per NeuronCore, weights
replicated; the LIF membrane state lives in SBUF per core so the T-step scan
needs no cross-device traffic.

Per-core network (per timestep): 5x [3x3 SAME conv + BN + LIF + 2x2 maxpool]
then FC(1024->256)+LIF, FC(256->110)+LIF, grouped mean (110->11), accumulated
over T in PSUM.

v2 pipeline (per conv-layer psum tile):
 - DVE STT primes psum with the LIF hard reset of the previous timestep's
   membrane state: pt <- (w < theta_prev) * w  (t=0 uses matmul start=True)
 - PE conv matmuls ACCUMULATE onto the primed psum (start=False), so the
   charge w_t = reset(w_{t-1}) + y_t happens inside the accumulator.
 - ACT drains psum -> w (bf16) with a DE-INTERLEAVED READ (even pixel
   columns first), which is free on the scalar engine and makes every
   downstream DVE op unit-stride.
 - DVE pool stage 1 (max of the two de-interleaved halves), then the spike
   threshold IS_GE on the contiguous q1 (spike-of-max == max-of-spikes),
   then pool stage 2 writes final spikes straight into the next layer's
   zero-padded spad (partition 64 holds the constant-1 bias row).
 - Membrane scaling w_t = 2^t * v_t (weights carry 2^(t-1)) keeps every
   LIF step exact in bf16; thresholds compare against 2^t.
GpSimd does one-time memsets only. No dup1 tile: conv1 runs the standard
9-tap accumulation like layers 2-4.
"""

import numpy as np

C = 64
T = 4
NL = 5
N_CORES = 8
BN_EPS = 1e-5
USE_BF16 = True

RES = [128, 64, 32, 16, 8]  # conv layer input resolution
PAIRED = [True, True, True, False, True]
PF_L = [1024, 1024, 512, 256, 32]  # psum tile free size per layer

_BUILT = {}


def _build_nc(debug=False):
    import concourse.bass as bass  # noqa: F401
    import concourse.mybir as mybir
    import concourse.tile as tile
    from concourse import bacc

    f32 = mybir.dt.float32
    mdt = mybir.dt.bfloat16 if USE_BF16 else f32
    Alu = mybir.AluOpType

    nc = bacc.Bacc(None, target_bir_lowering=False)

    xim_d = nc.dram_tensor("xim", [T, 19, 32 * 4, 130], mdt, kind="ExternalInput")
    w0_d = nc.dram_tensor("conv0T", [19, T * 64], mdt, kind="ExternalInput")
    wl_d = nc.dram_tensor("convsT", [4, 65, T * 576], mdt, kind="ExternalInput")
    c1p_d = nc.dram_tensor("conv1P", [128, T * 192], mdt, kind="ExternalInput")
    c2p_d = nc.dram_tensor("conv2P", [128, T * 192], mdt, kind="ExternalInput")
    f1_d = nc.dram_tensor("fc1k", [128, T * 2048], mdt, kind="ExternalInput")
    f2_d = nc.dram_tensor("fc2k", [128, T * 220], mdt, kind="ExternalInput")
    bb_d = nc.dram_tensor("boostB", [110, 11], f32, kind="ExternalInput")
    out_d = nc.dram_tensor("out", [1, 11], f32, kind="ExternalOutput")

    with tile.TileContext(nc) as tc:
        with (
            tc.tile_pool(name="const", bufs=1) as cpool,
            tc.tile_pool(name="state", bufs=1) as spool,
            tc.tile_pool(name="bands", bufs=8) as bpool,
            tc.tile_pool(name="work", bufs=4) as wpool,
            tc.tile_pool(name="cpsum", bufs=3, space="PSUM") as ppool,
            tc.tile_pool(name="fcpsum", bufs=1, space="PSUM") as pfc,
            tc.tile_pool(name="accpsum", bufs=1, space="PSUM") as pacc,
        ):
            # ---- constants ----
            w0 = cpool.tile([19, T * 64], mdt)
            nc.sync.dma_start(w0[:, :], w0_d[:, :])
            wl = []
            for l in range(1, 5):
                wt = cpool.tile([65, T * 576], mdt, name=f"wl{l}", tag=f"wl{l}")
                nc.sync.dma_start(wt[:, :], wl_d[l - 1])
                wl.append(wt)
            f1 = cpool.tile([128, T * 2048], mdt)
            for k in range(4):
                nc.sync.dma_start(
                    f1[:, k * 2048 : (k + 1) * 2048], f1_d[:, k * 2048 : (k + 1) * 2048]
                )
            f2 = cpool.tile([128, T * 220], mdt)
            nc.sync.dma_start(f2[:, :], f2_d[:, :])
            bb = cpool.tile([110, 11], f32)
            nc.sync.dma_start(bb[:, :], bb_d[:, :])
            c1p = cpool.tile([128, T * 192], mdt)
            nc.sync.dma_start(c1p[:, :], c1p_d[:, :])
            c2p = cpool.tile([128, T * 192], mdt)
            nc.sync.dma_start(c2p[:, :], c2p_d[:, :])

            # ---- state ----
            v = []
            for l in range(5):
                pp = 128 if PAIRED[l] else 64
                g = RES[l] * RES[l] // 2 if PAIRED[l] else RES[l] * RES[l]
                vt = spool.tile([pp, g], mdt, name=f"v{l}", tag=f"v{l}")
                v.append(vt)
            vf1 = spool.tile([128, 2], f32)
            nc.gpsimd.memset(vf1[:, :], 0.0)
            vf2 = spool.tile([110, 1], f32)
            nc.gpsimd.memset(vf2[:, :], 0.0)

            spads = [None]
            for l in range(1, 5):
                hp = RES[l] + 2
                sp = spool.tile([65, hp, hp], mdt, name=f"spad{l}", tag=f"spad{l}")
                nc.gpsimd.memset(sp[0:64, :, :], 0.0)
                nc.gpsimd.memset(sp[64:65, :, :], 1.0)
                spads.append(sp)

            # conv1/2 tap-pair tiles: partitions 0:64 = spad map, 64:128 = the
            # same map shifted one column left (covers dx=0,1 in one K=128
            # matmul); rebuilt per timestep by two SBUF->SBUF DMAs.
            dup1 = spool.tile([128, 66, 66], mdt)
            nc.gpsimd.memset(dup1[:, :, :], 0.0)
            dup2 = spool.tile([128, 34, 34], mdt)
            nc.gpsimd.memset(dup2[:, :, :], 0.0)
            dups = [None, dup1, dup2]
            cps = [None, c1p, c2p]

            s4p = spool.tile([128, 8], mdt)
            s1 = spool.tile([128, 2], mdt)
            s2 = spool.tile([110, 1], f32)

            acc_ps = pacc.tile([1, 11], f32)

            bands = {}

            def emit_dup(t, l):
                # 4 row-chunked DMAs per copy -> parallel queues/engines
                dp = dups[l]
                hp = RES[l] + 2
                q = hp // 4
                for k in range(4):
                    r0, r1 = k * q, (k + 1) * q if k < 3 else hp
                    nc.sync.dma_start(dp[0:64, r0:r1, :], spads[l][0:64, r0:r1, :])
                    nc.sync.dma_start(
                        dp[64:128, r0:r1, 0 : hp - 1], spads[l][0:64, r0:r1, 1:hp]
                    )

            def emit_bands(t):
                bs = []
                for bi in range(4):
                    bt = bpool.tile([19, 32, 130], mdt, name=f"band{bi}", tag="band")
                    nc.sync.dma_start(bt[0:10, :, :], xim_d[t, 0:10, bi * 32 : (bi + 1) * 32, :])
                    nc.sync.dma_start(bt[10:19, :, :], xim_d[t, 10:19, bi * 32 : (bi + 1) * 32, :])
                    bs.append(bt)
                bands[t] = bs

            def emit_layer(t, l):
                W = RES[l]
                paired = PAIRED[l]
                PF = PF_L[l]
                halfH = W // 2 if paired else W
                G = W * W // 2 if paired else W * W
                pp = 128 if paired else 64
                ntiles = G // PF
                rows_tile = PF // W
                banks = (PF + 511) // 512
                W2 = W // 2
                R2 = rows_tile // 2
                H2 = W // 2
                theta = float(2 ** t)
                theta_p = float(2 ** (t - 1))

                for ti in range(ntiles):
                    pt = ppool.tile([pp, PF], f32, name="cps", tag="cps")
                    vv = v[l][:, ti * PF : (ti + 1) * PF]
                    # view of vv as (two, x) and of pt in matching iteration
                    # order (psum natural pixel order; vv stores even pixels
                    # then odd pixels of each tile)
                    vv3 = vv.rearrange("p (two x) -> p two x", two=2)
                    pt3 = pt.rearrange("p (x two) -> p two x", two=2)
                    if t > 0:
                        # LIF hard reset of w_{t-1} primed into psum
                        nc.vector.scalar_tensor_tensor(
                            pt3[0:pp], vv3[0:pp], theta_p, vv3[0:pp], Alu.is_lt, Alu.mult
                        )
                    for b in range(banks):
                        cw = min(512, PF - b * 512)
                        rows_cw = cw // W
                        for half in range(2 if paired else 1):
                            r0 = (half * halfH if paired else 0) + ti * rows_tile + b * (512 // W)
                            oap = pt[64 * half : 64 * half + 64, b * 512 : b * 512 + cw]
                            if l == 0:
                                bi, r_loc = divmod(r0, 32)
                                nc.tensor.matmul(
                                    oap,
                                    w0[:, t * 64 : (t + 1) * 64],
                                    bands[t][bi][0:19, r_loc : r_loc + rows_cw, 0:128],
                                    start=(t == 0),
                                    stop=True,
                                    skip_group_check=True,
                                )
                            elif l <= 2 and t > 0:
                                for pi in range(6):
                                    if pi < 3:  # tap pair (dy,0)+(dy,1) via dup tile
                                        dy = pi
                                        nc.tensor.matmul(
                                            oap,
                                            cps[l][0:128, t * 192 + pi * 64 : t * 192 + pi * 64 + 64],
                                            dups[l][0:128, r0 + dy : r0 + dy + rows_cw, 0:W],
                                            start=(t == 0 and pi == 0),
                                            stop=False,
                                            skip_group_check=True,
                                        )
                                    else:  # singles (dy,2); last carries bias row
                                        dy = pi - 3
                                        p = dy * 3 + 2
                                        kp = 65 if p == 8 else 64
                                        nc.tensor.matmul(
                                            oap,
                                            wl[l - 1][0:kp, t * 576 + p * 64 : t * 576 + p * 64 + 64],
                                            spads[l][0:kp, r0 + dy : r0 + dy + rows_cw, 2 : 2 + W],
                                            start=False,
                                            stop=(pi == 5),
                                            skip_group_check=True,
                                        )
                            else:
                                for p in range(9):
                                    dy, dx = divmod(p, 3)
                                    kp = 65 if p == 8 else 64
                                    nc.tensor.matmul(
                                        oap,
                                        wl[l - 1][0:kp, t * 576 + p * 64 : t * 576 + p * 64 + 64],
                                        spads[l][0:kp, r0 + dy : r0 + dy + rows_cw, dx : dx + W],
                                        start=(t == 0 and p == 0),
                                        stop=(p == 8),
                                        skip_group_check=True,
                                    )
                    # ACT drain: w_t <- psum (de-interleaved read, unit write)
                    nc.scalar.copy(vv3[0:pp], pt3[0:pp])
                    # pool stage 1: max over pixel pairs == max of halves
                    q1 = wpool.tile([pp, PF // 2], mdt, name="q1", tag="q1")
                    nc.vector.tensor_tensor(
                        q1[:, :], vv[:, 0 : PF // 2], vv[:, PF // 2 : PF], Alu.max
                    )
                    # spike threshold on the contiguous q1 (in place)
                    nc.vector.tensor_scalar(q1[:, :], q1[:, :], theta, None, Alu.is_ge)
                    # pool stage 2: max over row pairs -> spikes into spad
                    q1r = q1.rearrange("p (r two w) -> p r two w", two=2, w=W2)
                    pr0 = ti * R2
                    if l < 4:
                        spn = spads[l + 1]
                        nc.vector.tensor_tensor(
                            spn[0:64, 1 + pr0 : 1 + pr0 + R2, 1 : 1 + W2],
                            q1r[0:64, :, 0, :], q1r[0:64, :, 1, :], Alu.max,
                        )
                        if paired:
                            hb = H2 // 2
                            nc.vector.tensor_tensor(
                                spn[0:64, 1 + hb + pr0 : 1 + hb + pr0 + R2, 1 : 1 + W2],
                                q1r[64:128, :, 0, :], q1r[64:128, :, 1, :], Alu.max,
                            )
                    else:
                        s4r = s4p.rearrange("p (r w) -> p r w", w=4)
                        nc.vector.tensor_tensor(
                            s4r[0:64, :, :], q1r[0:64, :, 0, :], q1r[0:64, :, 1, :], Alu.max
                        )
                        nc.vector.tensor_tensor(
                            s4r[64:128, :, :], q1r[64:128, :, 0, :], q1r[64:128, :, 1, :], Alu.max
                        )

            def emit_fc(t):
                theta = float(2 ** t)
                ps1 = pfc.tile([128, 2], f32, name="fcps", tag="fcps")
                for h in range(2):
                    for f in range(8):
                        nc.tensor.matmul(
                            ps1[0:128, h : h + 1],
                            f1[:, t * 2048 + (f * 2 + h) * 128 : t * 2048 + (f * 2 + h + 1) * 128],
                            s4p[:, f : f + 1],
                            start=(f == 0),
                            stop=(f == 7),
                        )
                nc.vector.scalar_tensor_tensor(
                    vf1[:, :], vf1[:, :], 1.0, ps1[0:128, 0:2], Alu.mult, Alu.add
                )
                nc.vector.tensor_scalar(s1[:, :], vf1[:, :], theta, None, Alu.is_ge)
                nc.vector.scalar_tensor_tensor(
                    vf1[:, :], vf1[:, :], theta, vf1[:, :], Alu.is_lt, Alu.mult
                )

                ps2 = pfc.tile([110, 1], f32, name="fcps2", tag="fcps")
                for h in range(2):
                    nc.tensor.matmul(
                        ps2[0:110, 0:1],
                        f2[:, t * 220 + h * 110 : t * 220 + (h + 1) * 110],
                        s1[:, h : h + 1],
                        start=(h == 0),
                        stop=(h == 1),
                    )
                nc.vector.scalar_tensor_tensor(
                    vf2[:, :], vf2[:, :], 1.0, ps2[0:110, 0:1], Alu.mult, Alu.add
                )
                nc.vector.tensor_scalar(s2[:, :], vf2[:, :], theta, None, Alu.is_ge)
                nc.vector.scalar_tensor_tensor(
                    vf2[:, :], vf2[:, :], theta, vf2[:, :], Alu.is_lt, Alu.mult
                )
                nc.tensor.matmul(
                    acc_ps[0:1, 0:11],
                    s2[0:110, 0:1],
                    bb[0:110, 0:11],
                    start=(t == 0),
                    stop=(t == T - 1),
                )

            emit_bands(0)
            emit_layer(0, 0)
            emit_layer(0, 1)
            for t in range(T):
                if t + 1 < T:
                    emit_bands(t + 1)
                    emit_layer(t + 1, 0)
                    emit_dup(t + 1, 1)
                for l in range(2, 5):
                    emit_layer(t, l)
                emit_fc(t)
                if t + 1 < T:
                    emit_layer(t + 1, 1)
                    emit_dup(t + 1, 2)

            out_sb = spool.tile([1, 11], f32)
            nc.scalar.copy(out_sb[:, :], acc_ps[0:1, 0:11])
            nc.sync.dma_start(out_d[0:1, 0:11], out_sb[:, :])

            if debug:
                dv0 = nc.dram_tensor("dbg_v0", [128, 8192], mdt, kind="ExternalOutput")
                nc.sync.dma_start(dv0[:, :], v[0][:, :])
                dv1 = nc.dram_tensor("dbg_v1", [128, 2048], mdt, kind="ExternalOutput")
                nc.sync.dma_start(dv1[:, :], v[1][:, :])
                dsp = nc.dram_tensor("dbg_spad1", [65, 66, 66], mdt, kind="ExternalOutput")
                nc.sync.dma_start(dsp[:, :, :], spads[1][:, :, :])

    nc.compile()
    return nc


def _prep_host(x, conv0_w, convs_w, bn_gamma, bn_beta, bn_mean, bn_var, fc1_w, fc2_w):
    f32 = np.float32
    x = np.asarray(x, f32)
    conv0_w = np.asarray(conv0_w, f32)
    convs_w = np.asarray(convs_w, f32)
    g = np.asarray(bn_gamma, f32) / np.sqrt(np.asarray(bn_var, f32) + BN_EPS)
    bconst = np.asarray(bn_beta, f32) - np.asarray(bn_mean, f32) * g
    fc1_w = np.asarray(fc1_w, f32)
    fc2_w = np.asarray(fc2_w, f32)

    n = x.shape[0]
    ts_scale = np.array([2.0 ** (t - 1) for t in range(T)], f32)

    conv0T = np.zeros((19, T * 64), f32)
    convsT = np.zeros((4, 65, T * 576), f32)
    conv1P = np.zeros((128, T * 192), f32)
    conv2P = np.zeros((128, T * 192), f32)
    for t in range(T):
        sc = ts_scale[t]
        c0 = slice(t * 64, (t + 1) * 64)
        for p in range(9):
            dy, dx = divmod(p, 3)
            for ci in range(2):
                conv0T[2 * p + ci, c0] = sc * g[0] * conv0_w[:, ci, dy, dx]
        conv0T[18, c0] = sc * bconst[0]
        for l in range(1, 5):
            for p in range(9):
                dy, dx = divmod(p, 3)
                convsT[l - 1, 0:64, t * 576 + p * 64 : t * 576 + (p + 1) * 64] = (
                    sc * g[l][None, :] * convs_w[l - 1][:, :, dy, dx].T
                )
            convsT[l - 1, 64, t * 576 + 8 * 64 : t * 576 + 9 * 64] = sc * bconst[l]
        for dy in range(3):
            for li, cpx in ((1, conv1P), (2, conv2P)):
                cpx[0:64, t * 192 + dy * 64 : t * 192 + (dy + 1) * 64] = (
                    sc * g[li][None, :] * convs_w[li - 1][:, :, dy, 0].T
                )
                cpx[64:128, t * 192 + dy * 64 : t * 192 + (dy + 1) * 64] = (
                    sc * g[li][None, :] * convs_w[li - 1][:, :, dy, 1].T
                )

    xpad = np.zeros((n, T, 2, 130, 130), f32)
    xpad[:, :, :, 1:129, 1:129] = x
    xim = np.zeros((n, T, 19, 130, 130), f32)
    for p in range(9):
        dy, dx = divmod(p, 3)
        for ci in range(2):
            xim[:, :, 2 * p + ci, 0:128, 0:128] = xpad[:, :, ci, dy : dy + 128, dx : dx + 128]
    xim[:, :, 18] = 1.0
    xim = np.ascontiguousarray(xim[:, :, :, 0:128, :])

    p_idx = np.arange(128)
    fc1k = np.zeros((128, T * 2048), f32)
    fc2k = np.zeros((128, T * 220), f32)
    for t in range(T):
        sc = ts_scale[t]
        for f in range(8):
            kcol = (p_idx % 64) * 16 + (p_idx // 64) * 8 + f
            for h in range(2):
                fc1k[:, t * 2048 + (f * 2 + h) * 128 : t * 2048 + (f * 2 + h + 1) * 128] = (
                    sc * fc1_w[h * 128 : (h + 1) * 128, kcol].T
                )
        for h in range(2):
            fc2k[:, t * 220 + h * 110 : t * 220 + (h + 1) * 110] = (
                sc * fc2_w[:, h * 128 : (h + 1) * 128].T
            )

    boostB = np.zeros((110, 11), f32)
    for k in range(110):
        boostB[k, k // 10] = 0.1

    if USE_BF16:
        import ml_dtypes

        bf16 = ml_dtypes.bfloat16
        xim, conv0T, convsT, conv1P, conv2P, fc1k, fc2k = (
            a.astype(bf16) for a in (xim, conv0T, convsT, conv1P, conv2P, fc1k, fc2k)
        )
    return xim, conv0T, convsT, conv1P, conv2P, fc1k, fc2k, boostB


def kernel(**inputs):
    import os

    from concourse.bass_utils import run_bass_kernel_spmd

    debug = bool(int(os.environ.get("KERNEL_DEBUG", "0")))

    x = np.asarray(inputs["x"], np.float32)
    assert x.shape == (8, 4, 2, 128, 128), x.shape
    xim, conv0T, convsT, conv1P, conv2P, fc1k, fc2k, boostB = _prep_host(
        x,
        inputs["conv0_w"],
        inputs["convs_w"],
        inputs["bn_gamma"],
        inputs["bn_beta"],
        inputs["bn_mean"],
        inputs["bn_var"],
        inputs["fc1_w"],
        inputs["fc2_w"],
    )

    if debug not in _BUILT:
        _BUILT[debug] = _build_nc(debug)
    nc = _BUILT[debug]

    shared = dict(conv0T=conv0T, convsT=convsT, conv1P=conv1P, conv2P=conv2P, fc1k=fc1k, fc2k=fc2k, boostB=boostB)
    in_maps = [dict(xim=np.ascontiguousarray(xim[n]), **shared) for n in range(N_CORES)]
    res = run_bass_kernel_spmd(nc, in_maps, core_ids=list(range(N_CORES)))
    global LAST_RESULT
    LAST_RESULT = res
    return np.stack([res.results[n]["out"][0] for n in range(N_CORES)], axis=0)


# revision 29
# speedup vs baseline: 1.0181x; 1.0060x over previous
"""Trainium2 Bass kernel: DVS128-gesture spiking CNN inference (batch 8, T=4).

Sharding: data-parallel over batch N=8 -> 1 sample per NeuronCore, weights
replicated; the LIF membrane state lives in SBUF per core so the T-step scan
needs no cross-device traffic.

Per-core network (per timestep): 5x [3x3 SAME conv + BN + LIF + 2x2 maxpool]
then FC(1024->256)+LIF, FC(256->110)+LIF, grouped mean (110->11), accumulated
over T in PSUM.

v2 pipeline (per conv-layer psum tile):
 - DVE STT primes psum with the LIF hard reset of the previous timestep's
   membrane state: pt <- (w < theta_prev) * w  (t=0 uses matmul start=True)
 - PE conv matmuls ACCUMULATE onto the primed psum (start=False), so the
   charge w_t = reset(w_{t-1}) + y_t happens inside the accumulator.
 - ACT drains psum -> w (bf16) with a DE-INTERLEAVED READ (even pixel
   columns first), which is free on the scalar engine and makes every
   downstream DVE op unit-stride.
 - DVE pool stage 1 (max of the two de-interleaved halves), then the spike
   threshold IS_GE on the contiguous q1 (spike-of-max == max-of-spikes),
   then pool stage 2 writes final spikes straight into the next layer's
   zero-padded spad (partition 64 holds the constant-1 bias row).
 - Membrane scaling w_t = 2^t * v_t (weights carry 2^(t-1)) keeps every
   LIF step exact in bf16; thresholds compare against 2^t.
GpSimd does one-time memsets only. No dup1 tile: conv1 runs the standard
9-tap accumulation like layers 2-4.
"""

import numpy as np

C = 64
T = 4
NL = 5
N_CORES = 8
BN_EPS = 1e-5
USE_BF16 = True

RES = [128, 64, 32, 16, 8]  # conv layer input resolution
PAIRED = [True, True, True, False, True]
PF_L = [1024, 1024, 512, 256, 32]  # psum tile free size per layer

_BUILT = {}


def _build_nc(debug=False):
    import concourse.bass as bass  # noqa: F401
    import concourse.mybir as mybir
    import concourse.tile as tile
    from concourse import bacc

    f32 = mybir.dt.float32
    mdt = mybir.dt.bfloat16 if USE_BF16 else f32
    Alu = mybir.AluOpType

    nc = bacc.Bacc(None, target_bir_lowering=False)

    xim_d = nc.dram_tensor("xim", [T, 19, 32 * 4, 130], mdt, kind="ExternalInput")
    w0_d = nc.dram_tensor("conv0T", [19, T * 64], mdt, kind="ExternalInput")
    wl_d = nc.dram_tensor("convsT", [4, 65, T * 576], mdt, kind="ExternalInput")
    c1p_d = nc.dram_tensor("conv1P", [128, T * 192], mdt, kind="ExternalInput")
    c2p_d = nc.dram_tensor("conv2P", [128, T * 192], mdt, kind="ExternalInput")
    f1_d = nc.dram_tensor("fc1k", [128, T * 2048], mdt, kind="ExternalInput")
    f2_d = nc.dram_tensor("fc2k", [128, T * 220], mdt, kind="ExternalInput")
    bb_d = nc.dram_tensor("boostB", [110, 11], f32, kind="ExternalInput")
    out_d = nc.dram_tensor("out", [1, 11], f32, kind="ExternalOutput")

    with tile.TileContext(nc) as tc:
        with (
            tc.tile_pool(name="const", bufs=1) as cpool,
            tc.tile_pool(name="state", bufs=1) as spool,
            tc.tile_pool(name="bands", bufs=8) as bpool,
            tc.tile_pool(name="work", bufs=4) as wpool,
            tc.tile_pool(name="cpsum", bufs=3, space="PSUM") as ppool,
            tc.tile_pool(name="fcpsum", bufs=1, space="PSUM") as pfc,
            tc.tile_pool(name="accpsum", bufs=1, space="PSUM") as pacc,
        ):
            # ---- constants ----
            w0 = cpool.tile([19, T * 64], mdt)
            nc.sync.dma_start(w0[:, :], w0_d[:, :])
            wl = []
            for l in range(1, 5):
                wt = cpool.tile([65, T * 576], mdt, name=f"wl{l}", tag=f"wl{l}")
                nc.sync.dma_start(wt[:, :], wl_d[l - 1])
                wl.append(wt)
            f1 = cpool.tile([128, T * 2048], mdt)
            for k in range(4):
                nc.sync.dma_start(
                    f1[:, k * 2048 : (k + 1) * 2048], f1_d[:, k * 2048 : (k + 1) * 2048]
                )
            f2 = cpool.tile([128, T * 220], mdt)
            nc.sync.dma_start(f2[:, :], f2_d[:, :])
            bb = cpool.tile([110, 11], f32)
            nc.sync.dma_start(bb[:, :], bb_d[:, :])
            c1p = cpool.tile([128, T * 192], mdt)
            nc.sync.dma_start(c1p[:, :], c1p_d[:, :])
            c2p = cpool.tile([128, T * 192], mdt)
            nc.sync.dma_start(c2p[:, :], c2p_d[:, :])

            # ---- state ----
            v = []
            for l in range(5):
                pp = 128 if PAIRED[l] else 64
                g = RES[l] * RES[l] // 2 if PAIRED[l] else RES[l] * RES[l]
                vt = spool.tile([pp, g], mdt, name=f"v{l}", tag=f"v{l}")
                v.append(vt)
            vf1 = spool.tile([128, 2], f32)
            nc.gpsimd.memset(vf1[:, :], 0.0)
            vf2 = spool.tile([110, 1], f32)
            nc.gpsimd.memset(vf2[:, :], 0.0)

            spads = [None]
            for l in range(1, 5):
                hp = RES[l] + 2
                sp = spool.tile([65, hp, hp], mdt, name=f"spad{l}", tag=f"spad{l}")
                nc.gpsimd.memset(sp[0:64, :, :], 0.0)
                nc.gpsimd.memset(sp[64:65, :, :], 1.0)
                spads.append(sp)

            # conv1/2 tap-pair tiles: partitions 0:64 = spad map, 64:128 = the
            # same map shifted one column left (covers dx=0,1 in one K=128
            # matmul); rebuilt per timestep by two SBUF->SBUF DMAs.
            dup1 = spool.tile([128, 66, 66], mdt)
            nc.gpsimd.memset(dup1[:, :, :], 0.0)
            dups = [None, dup1]
            cps = [None, c1p]

            s4p = spool.tile([128, 8], mdt)
            s1 = spool.tile([128, 2], mdt)
            s2 = spool.tile([110, 1], f32)

            acc_ps = pacc.tile([1, 11], f32)

            bands = {}

            def emit_dup(t, l):
                # 4 row-chunked DMAs per copy -> parallel queues/engines
                dp = dups[l]
                hp = RES[l] + 2
                q = hp // 4
                for k in range(4):
                    r0, r1 = k * q, (k + 1) * q if k < 3 else hp
                    nc.sync.dma_start(dp[0:64, r0:r1, :], spads[l][0:64, r0:r1, :])
                    nc.sync.dma_start(
                        dp[64:128, r0:r1, 0 : hp - 1], spads[l][0:64, r0:r1, 1:hp]
                    )

            def emit_bands(t):
                bs = []
                for bi in range(4):
                    bt = bpool.tile([19, 32, 130], mdt, name=f"band{bi}", tag="band")
                    nc.sync.dma_start(bt[0:10, :, :], xim_d[t, 0:10, bi * 32 : (bi + 1) * 32, :])
                    nc.sync.dma_start(bt[10:19, :, :], xim_d[t, 10:19, bi * 32 : (bi + 1) * 32, :])
                    bs.append(bt)
                bands[t] = bs

            def emit_layer(t, l):
                W = RES[l]
                paired = PAIRED[l]
                PF = PF_L[l]
                halfH = W // 2 if paired else W
                G = W * W // 2 if paired else W * W
                pp = 128 if paired else 64
                ntiles = G // PF
                rows_tile = PF // W
                banks = (PF + 511) // 512
                W2 = W // 2
                R2 = rows_tile // 2
                H2 = W // 2
                theta = float(2 ** t)
                theta_p = float(2 ** (t - 1))

                for ti in range(ntiles):
                    pt = ppool.tile([pp, PF], f32, name="cps", tag="cps")
                    vv = v[l][:, ti * PF : (ti + 1) * PF]
                    # view of vv as (two, x) and of pt in matching iteration
                    # order (psum natural pixel order; vv stores even pixels
                    # then odd pixels of each tile)
                    vv3 = vv.rearrange("p (two x) -> p two x", two=2)
                    pt3 = pt.rearrange("p (x two) -> p two x", two=2)
                    if t > 0:
                        # LIF hard reset of w_{t-1} primed into psum
                        nc.vector.scalar_tensor_tensor(
                            pt3[0:pp], vv3[0:pp], theta_p, vv3[0:pp], Alu.is_lt, Alu.mult
                        )
                    for b in range(banks):
                        cw = min(512, PF - b * 512)
                        rows_cw = cw // W
                        for half in range(2 if paired else 1):
                            r0 = (half * halfH if paired else 0) + ti * rows_tile + b * (512 // W)
                            oap = pt[64 * half : 64 * half + 64, b * 512 : b * 512 + cw]
                            if l == 0:
                                bi, r_loc = divmod(r0, 32)
                                nc.tensor.matmul(
                                    oap,
                                    w0[:, t * 64 : (t + 1) * 64],
                                    bands[t][bi][0:19, r_loc : r_loc + rows_cw, 0:128],
                                    start=(t == 0),
                                    stop=True,
                                    skip_group_check=True,
                                )
                            elif l == 1 and t > 0:
                                for pi in range(6):
                                    if pi < 3:  # tap pair (dy,0)+(dy,1) via dup tile
                                        dy = pi
                                        nc.tensor.matmul(
                                            oap,
                                            cps[l][0:128, t * 192 + pi * 64 : t * 192 + pi * 64 + 64],
                                            dups[l][0:128, r0 + dy : r0 + dy + rows_cw, 0:W],
                                            start=(t == 0 and pi == 0),
                                            stop=False,
                                            skip_group_check=True,
                                        )
                                    else:  # singles (dy,2); last carries bias row
                                        dy = pi - 3
                                        p = dy * 3 + 2
                                        kp = 65 if p == 8 else 64
                                        nc.tensor.matmul(
                                            oap,
                                            wl[l - 1][0:kp, t * 576 + p * 64 : t * 576 + p * 64 + 64],
                                            spads[l][0:kp, r0 + dy : r0 + dy + rows_cw, 2 : 2 + W],
                                            start=False,
                                            stop=(pi == 5),
                                            skip_group_check=True,
                                        )
                            else:
                                for p in range(9):
                                    dy, dx = divmod(p, 3)
                                    kp = 65 if p == 8 else 64
                                    nc.tensor.matmul(
                                        oap,
                                        wl[l - 1][0:kp, t * 576 + p * 64 : t * 576 + p * 64 + 64],
                                        spads[l][0:kp, r0 + dy : r0 + dy + rows_cw, dx : dx + W],
                                        start=(t == 0 and p == 0),
                                        stop=(p == 8),
                                        skip_group_check=True,
                                    )
                    # ACT drain: w_t <- psum (de-interleaved read, unit write)
                    nc.scalar.copy(vv3[0:pp], pt3[0:pp])
                    # pool stage 1: max over pixel pairs == max of halves
                    q1 = wpool.tile([pp, PF // 2], mdt, name="q1", tag="q1")
                    nc.vector.tensor_tensor(
                        q1[:, :], vv[:, 0 : PF // 2], vv[:, PF // 2 : PF], Alu.max
                    )
                    # spike threshold on the contiguous q1 (in place)
                    nc.vector.tensor_scalar(q1[:, :], q1[:, :], theta, None, Alu.is_ge)
                    # pool stage 2: max over row pairs -> spikes into spad
                    q1r = q1.rearrange("p (r two w) -> p r two w", two=2, w=W2)
                    pr0 = ti * R2
                    if l < 4:
                        spn = spads[l + 1]
                        nc.vector.tensor_tensor(
                            spn[0:64, 1 + pr0 : 1 + pr0 + R2, 1 : 1 + W2],
                            q1r[0:64, :, 0, :], q1r[0:64, :, 1, :], Alu.max,
                        )
                        if paired:
                            hb = H2 // 2
                            nc.vector.tensor_tensor(
                                spn[0:64, 1 + hb + pr0 : 1 + hb + pr0 + R2, 1 : 1 + W2],
                                q1r[64:128, :, 0, :], q1r[64:128, :, 1, :], Alu.max,
                            )
                    else:
                        s4r = s4p.rearrange("p (r w) -> p r w", w=4)
                        nc.vector.tensor_tensor(
                            s4r[0:64, :, :], q1r[0:64, :, 0, :], q1r[0:64, :, 1, :], Alu.max
                        )
                        nc.vector.tensor_tensor(
                            s4r[64:128, :, :], q1r[64:128, :, 0, :], q1r[64:128, :, 1, :], Alu.max
                        )

            def emit_fc(t):
                theta = float(2 ** t)
                ps1 = pfc.tile([128, 2], f32, name="fcps", tag="fcps")
                for h in range(2):
                    for f in range(8):
                        nc.tensor.matmul(
                            ps1[0:128, h : h + 1],
                            f1[:, t * 2048 + (f * 2 + h) * 128 : t * 2048 + (f * 2 + h + 1) * 128],
                            s4p[:, f : f + 1],
                            start=(f == 0),
                            stop=(f == 7),
                        )
                nc.vector.scalar_tensor_tensor(
                    vf1[:, :], vf1[:, :], 1.0, ps1[0:128, 0:2], Alu.mult, Alu.add
                )
                nc.vector.tensor_scalar(s1[:, :], vf1[:, :], theta, None, Alu.is_ge)
                nc.vector.scalar_tensor_tensor(
                    vf1[:, :], vf1[:, :], theta, vf1[:, :], Alu.is_lt, Alu.mult
                )

                ps2 = pfc.tile([110, 1], f32, name="fcps2", tag="fcps")
                for h in range(2):
                    nc.tensor.matmul(
                        ps2[0:110, 0:1],
                        f2[:, t * 220 + h * 110 : t * 220 + (h + 1) * 110],
                        s1[:, h : h + 1],
                        start=(h == 0),
                        stop=(h == 1),
                    )
                nc.vector.scalar_tensor_tensor(
                    vf2[:, :], vf2[:, :], 1.0, ps2[0:110, 0:1], Alu.mult, Alu.add
                )
                nc.vector.tensor_scalar(s2[:, :], vf2[:, :], theta, None, Alu.is_ge)
                nc.vector.scalar_tensor_tensor(
                    vf2[:, :], vf2[:, :], theta, vf2[:, :], Alu.is_lt, Alu.mult
                )
                nc.tensor.matmul(
                    acc_ps[0:1, 0:11],
                    s2[0:110, 0:1],
                    bb[0:110, 0:11],
                    start=(t == 0),
                    stop=(t == T - 1),
                )

            emit_bands(0)
            emit_layer(0, 0)
            emit_layer(0, 1)
            for t in range(T):
                if t + 1 < T:
                    emit_bands(t + 1)
                    emit_layer(t + 1, 0)
                    emit_dup(t + 1, 1)
                for l in range(2, 5):
                    emit_layer(t, l)
                emit_fc(t)
                if t + 1 < T:
                    emit_layer(t + 1, 1)

            out_sb = spool.tile([1, 11], f32)
            nc.scalar.copy(out_sb[:, :], acc_ps[0:1, 0:11])
            nc.sync.dma_start(out_d[0:1, 0:11], out_sb[:, :])

            if debug:
                dv0 = nc.dram_tensor("dbg_v0", [128, 8192], mdt, kind="ExternalOutput")
                nc.sync.dma_start(dv0[:, :], v[0][:, :])
                dv1 = nc.dram_tensor("dbg_v1", [128, 2048], mdt, kind="ExternalOutput")
                nc.sync.dma_start(dv1[:, :], v[1][:, :])
                dsp = nc.dram_tensor("dbg_spad1", [65, 66, 66], mdt, kind="ExternalOutput")
                nc.sync.dma_start(dsp[:, :, :], spads[1][:, :, :])

    nc.compile()
    return nc


def _prep_host(x, conv0_w, convs_w, bn_gamma, bn_beta, bn_mean, bn_var, fc1_w, fc2_w):
    f32 = np.float32
    x = np.asarray(x, f32)
    conv0_w = np.asarray(conv0_w, f32)
    convs_w = np.asarray(convs_w, f32)
    g = np.asarray(bn_gamma, f32) / np.sqrt(np.asarray(bn_var, f32) + BN_EPS)
    bconst = np.asarray(bn_beta, f32) - np.asarray(bn_mean, f32) * g
    fc1_w = np.asarray(fc1_w, f32)
    fc2_w = np.asarray(fc2_w, f32)

    n = x.shape[0]
    ts_scale = np.array([2.0 ** (t - 1) for t in range(T)], f32)

    conv0T = np.zeros((19, T * 64), f32)
    convsT = np.zeros((4, 65, T * 576), f32)
    conv1P = np.zeros((128, T * 192), f32)
    conv2P = np.zeros((128, T * 192), f32)
    for t in range(T):
        sc = ts_scale[t]
        c0 = slice(t * 64, (t + 1) * 64)
        for p in range(9):
            dy, dx = divmod(p, 3)
            for ci in range(2):
                conv0T[2 * p + ci, c0] = sc * g[0] * conv0_w[:, ci, dy, dx]
        conv0T[18, c0] = sc * bconst[0]
        for l in range(1, 5):
            for p in range(9):
                dy, dx = divmod(p, 3)
                convsT[l - 1, 0:64, t * 576 + p * 64 : t * 576 + (p + 1) * 64] = (
                    sc * g[l][None, :] * convs_w[l - 1][:, :, dy, dx].T
                )
            convsT[l - 1, 64, t * 576 + 8 * 64 : t * 576 + 9 * 64] = sc * bconst[l]
        for dy in range(3):
            for li, cpx in ((1, conv1P), (2, conv2P)):
                cpx[0:64, t * 192 + dy * 64 : t * 192 + (dy + 1) * 64] = (
                    sc * g[li][None, :] * convs_w[li - 1][:, :, dy, 0].T
                )
                cpx[64:128, t * 192 + dy * 64 : t * 192 + (dy + 1) * 64] = (
                    sc * g[li][None, :] * convs_w[li - 1][:, :, dy, 1].T
                )

    xpad = np.zeros((n, T, 2, 130, 130), f32)
    xpad[:, :, :, 1:129, 1:129] = x
    xim = np.zeros((n, T, 19, 130, 130), f32)
    for p in range(9):
        dy, dx = divmod(p, 3)
        for ci in range(2):
            xim[:, :, 2 * p + ci, 0:128, 0:128] = xpad[:, :, ci, dy : dy + 128, dx : dx + 128]
    xim[:, :, 18] = 1.0
    xim = np.ascontiguousarray(xim[:, :, :, 0:128, :])

    p_idx = np.arange(128)
    fc1k = np.zeros((128, T * 2048), f32)
    fc2k = np.zeros((128, T * 220), f32)
    for t in range(T):
        sc = ts_scale[t]
        for f in range(8):
            kcol = (p_idx % 64) * 16 + (p_idx // 64) * 8 + f
            for h in range(2):
                fc1k[:, t * 2048 + (f * 2 + h) * 128 : t * 2048 + (f * 2 + h + 1) * 128] = (
                    sc * fc1_w[h * 128 : (h + 1) * 128, kcol].T
                )
        for h in range(2):
            fc2k[:, t * 220 + h * 110 : t * 220 + (h + 1) * 110] = (
                sc * fc2_w[:, h * 128 : (h + 1) * 128].T
            )

    boostB = np.zeros((110, 11), f32)
    for k in range(110):
        boostB[k, k // 10] = 0.1

    if USE_BF16:
        import ml_dtypes

        bf16 = ml_dtypes.bfloat16
        xim, conv0T, convsT, conv1P, conv2P, fc1k, fc2k = (
            a.astype(bf16) for a in (xim, conv0T, convsT, conv1P, conv2P, fc1k, fc2k)
        )
    return xim, conv0T, convsT, conv1P, conv2P, fc1k, fc2k, boostB


def kernel(**inputs):
    import os

    from concourse.bass_utils import run_bass_kernel_spmd

    debug = bool(int(os.environ.get("KERNEL_DEBUG", "0")))

    x = np.asarray(inputs["x"], np.float32)
    assert x.shape == (8, 4, 2, 128, 128), x.shape
    xim, conv0T, convsT, conv1P, conv2P, fc1k, fc2k, boostB = _prep_host(
        x,
        inputs["conv0_w"],
        inputs["convs_w"],
        inputs["bn_gamma"],
        inputs["bn_beta"],
        inputs["bn_mean"],
        inputs["bn_var"],
        inputs["fc1_w"],
        inputs["fc2_w"],
    )

    if debug not in _BUILT:
        _BUILT[debug] = _build_nc(debug)
    nc = _BUILT[debug]

    shared = dict(conv0T=conv0T, convsT=convsT, conv1P=conv1P, conv2P=conv2P, fc1k=fc1k, fc2k=fc2k, boostB=boostB)
    in_maps = [dict(xim=np.ascontiguousarray(xim[n]), **shared) for n in range(N_CORES)]
    res = run_bass_kernel_spmd(nc, in_maps, core_ids=list(range(N_CORES)))
    global LAST_RESULT
    LAST_RESULT = res
    return np.stack([res.results[n]["out"][0] for n in range(N_CORES)], axis=0)
